# revision 15
# baseline (speedup 1.0000x reference)
"""HGNN message passing (gather + segment_sum + residual) on 8 trn2 cores.

out = x + segment_sum(x[src_idx], dst_idx, num_segments=N)

Strategy (node-sharded accumulation, no collectives):
  - dst nodes sharded across 8 cores (12500 nodes each); each core owns the
    edges targeting its node range and produces its [12500, 128] output slice.
  - Nodes are processed in 50 PAIRS of 125-node blocks (A, B). Edges of a
    pair are bucketed by src//25000 (4 buckets, int16 gather-offset reach)
    and tightly packed: block A's edges first, then block B's, then a -1
    tail the Q7 gather kernel never touches (the count register carries the
    exact edge count, so descriptor generation is O(edges) with no padding).
  - f32 x rows (512B) are fetched with gpsimd dma_gather across the 4 SWDGE
    queues (4 Q7 core-pairs generating descriptors in parallel), then cast
    to bf16 on the Scalar (ACT) engine.
  - Per pair, ONE fused DVE is_equal builds all one-hot matrices at once
    (dstv column broadcast against a static iota image); the segment-sum is
    a sum of bf16 one-hot matmuls accumulated in PSUM (one PSUM tile per
    block). Chunks that can straddle the A/B edge boundary get one matmul
    per side (the host writes -5 into the other side's dstv so the one-hot
    is zero there). The residual enters the same PSUM accumulation as an
    identity-matrix matmul against the bf16 x row block, and ACT copies
    PSUM to SBUF for the output DMA.

All cores run one SPMD program; per-core data differences live entirely in
the input tensors (edge indices, dst values, residual slices). The matmul
template (chunk count, boundary-chunk span) is computed from the actual
edge data at build time but is identical across cores.
"""
import os

import numpy as np

N_NODES = 100000
D = 128
N_CORES = 8
NODES_PER_CORE = N_NODES // N_CORES  # 12500
BLOCK = 125
NBLOCKS = NODES_PER_CORE // BLOCK  # 100
if os.environ.get("KERNEL_NBLOCKS"):  # debug-only scale-down (even values)
    NBLOCKS = int(os.environ["KERNEL_NBLOCKS"])
NPAIRS = NBLOCKS // 2
NBKT = 4
SRC_CHUNK = N_NODES // NBKT  # 25000
NGATH = NPAIRS * NBKT  # gathers per core
STAGE_BUFS = 8

_cached = {}


def _build_program(chunks_p, bmin, bmax):
    """chunks_p: slots per (pair,bucket) gather / 128; the A/B boundary chunk
    is always in [bmin, bmax] (host-verified at preprocess time)."""
    from concourse import bacc, mybir, library_config
    import concourse.tile as tile

    capp = chunks_p * 128
    ncols_a = bmax + 1
    nc_pb = ncols_a + (chunks_p - bmin)  # dstv/onehot columns per (pair, bucket)
    nc_pair = NBKT * nc_pb
    idx_cols = NGATH * (capp // 16)

    nc = bacc.Bacc("TRN2", debug=False, num_swdge_queues=4)
    f32 = mybir.dt.float32
    bf16 = mybir.dt.bfloat16
    x_t = nc.dram_tensor("x", [N_NODES, D], bf16, kind="ExternalInput")
    xresb_t = nc.dram_tensor("xresb", [NBLOCKS * BLOCK, D], bf16, kind="ExternalInput")
    idx_t = nc.dram_tensor("idx", [128, idx_cols], mybir.dt.int16, kind="ExternalInput")
    cnt_t = nc.dram_tensor("cnt", [1, NGATH], mybir.dt.int32, kind="ExternalInput")
    dstv_t = nc.dram_tensor("dstv", [128, NPAIRS * nc_pair], f32, kind="ExternalInput")
    iota_t = nc.dram_tensor("iota", [128, nc_pair, BLOCK], bf16, kind="ExternalInput")
    ident_t = nc.dram_tensor("ident", [128, BLOCK], bf16, kind="ExternalInput")
    out_t = nc.dram_tensor("out", [NBLOCKS * BLOCK, D], f32, kind="ExternalOutput")

    with tile.TileContext(nc) as tc:
        with (
            tc.tile_pool(name="consts", bufs=1) as constp,
            tc.tile_pool(name="stage", bufs=STAGE_BUFS) as stagep,
            tc.tile_pool(name="stageb", bufs=STAGE_BUFS) as stagebp,
            tc.tile_pool(name="oh", bufs=3) as ohp,
            tc.tile_pool(name="psum", bufs=4, space="PSUM") as psump,
            tc.tile_pool(name="resid", bufs=4) as residp,
            tc.tile_pool(name="osb", bufs=4) as osbp,
        ):
            nc.gpsimd.load_library(library_config.mlp)
            idx_sb = constp.tile([128, idx_cols], mybir.dt.int16)
            nc.sync.dma_start(idx_sb[:], idx_t[:])
            cnt_sb = constp.tile([1, NGATH], mybir.dt.int32)
            nc.sync.dma_start(cnt_sb[:], cnt_t[:])
            cnt_regs = [nc.gpsimd.alloc_register(f"cnt{k}") for k in range(NBKT)]
            dstv_sb = constp.tile([128, NPAIRS * nc_pair], f32)
            nc.sync.dma_start(dstv_sb[:], dstv_t[:])
            iota_sb = constp.tile([128, nc_pair, BLOCK], bf16)
            nc.sync.dma_start(iota_sb[:], iota_t[:])
            ident_sb = constp.tile([128, BLOCK], bf16)
            nc.sync.dma_start(ident_sb[:], ident_t[:])

            # zero staging/residual once: stale SBUF may hold NaN bit
            # patterns, and NaN * 0 would poison the PSUM accumulation
            for _ in range(STAGE_BUFS):
                stage = stagep.tile([128, chunks_p, D], bf16)
                nc.vector.memset(stage[:], 0.0)
            resid_bufs = []
            for _ in range(4):
                resid = residp.tile([128, D], bf16)
                nc.vector.memset(resid[:], 0.0)
                resid_bufs.append(resid)

            # (half, chunk) template, identical for every (pair, bucket)
            cols = [("A", c) for c in range(ncols_a)] + [
                ("B", c) for c in range(bmin, chunks_p)
            ]

            for p in range(NPAIRS):
                stages_p = []
                for k in range(NBKT):
                    g = p * NBKT + k
                    stage = stagep.tile([128, chunks_p, D], bf16)
                    nc.gpsimd.reg_load(cnt_regs[k], cnt_sb[:1, g : g + 1])
                    nc.gpsimd.dma_gather(
                        stage[:],
                        x_t[k * SRC_CHUNK : (k + 1) * SRC_CHUNK, :],
                        idx_sb[:, g * (capp // 16) : (g + 1) * (capp // 16)],
                        capp,
                        cnt_regs[k],
                        D,
                        single_packet=False,
                        queue_num=k,
                    )
                    stages_p.append(stage)

                ohb = ohp.tile([128, nc_pair, BLOCK], bf16)
                dstv_b = (
                    dstv_sb[:, p * nc_pair : (p + 1) * nc_pair]
                    .unsqueeze(2)
                    .broadcast_to([128, nc_pair, BLOCK])
                )
                nc.vector.tensor_tensor(
                    ohb[:], dstv_b, iota_sb[:], mybir.AluOpType.is_equal
                )

                for h, half in enumerate(("A", "B")):
                    b = 2 * p + h
                    resid = resid_bufs[b % 4]
                    nc.sync.dma_start(
                        resid[:BLOCK], xresb_t[b * BLOCK : (b + 1) * BLOCK, :]
                    )
                    psum = psump.tile([BLOCK, D], f32, space="PSUM")
                    nc.tensor.matmul(
                        out=psum[:],
                        lhsT=ident_sb[:],
                        rhs=resid[:],
                        start=True,
                        stop=False,
                    )
                    hcols = [(j, c) for j, (hh, c) in enumerate(cols) if hh == half]
                    for k in range(NBKT):
                        for i, (j, c) in enumerate(hcols):
                            nc.tensor.matmul(
                                out=psum[:],
                                lhsT=ohb[:, k * nc_pb + j, :],
                                rhs=stages_p[k][:, c, :],
                                start=False,
                                stop=(k == NBKT - 1 and i == len(hcols) - 1),
                            )
                    osb = osbp.tile([BLOCK, D], f32)
                    nc.scalar.copy(osb[:], psum[:])
                    nc.sync.dma_start(out_t[b * BLOCK : (b + 1) * BLOCK, :], osb[:])

    nc.compile()
    return nc


def _preprocess(src, dst):
    """Pack edges into tight per-(pair,bucket) gather regions; build the idx
    image, exact counts, and the dstv one-hot source columns."""
    src = src.astype(np.int64)
    dst = dst.astype(np.int64)
    if NBLOCKS < NODES_PER_CORE // BLOCK:  # debug: drop edges past the cut
        keep = (dst % NODES_PER_CORE) // BLOCK < NBLOCKS
        src, dst = src[keep], dst[keep]
    E = src.shape[0]
    core = dst // NODES_PER_CORE
    blk = (dst % NODES_PER_CORE) // BLOCK
    half = blk % 2
    pair = blk // 2
    dloc = (dst % NODES_PER_CORE) % BLOCK
    bkt = src // SRC_CHUNK
    sloc = src % SRC_CHUNK
    region = (core * NPAIRS + pair) * NBKT + bkt  # gather region id
    tot_reg = N_CORES * NGATH

    key = region * 2 + half
    order = np.argsort(key, kind="stable")
    ks = key[order]
    counts2 = np.bincount(key, minlength=tot_reg * 2)
    starts2 = np.zeros(tot_reg * 2 + 1, np.int64)
    np.cumsum(counts2, out=starts2[1:])
    within = np.arange(E, dtype=np.int64) - starts2[ks]

    cnt_a = counts2[0::2]
    cnt_tot = counts2[0::2] + counts2[1::2]
    # slot within region: A edges first, then B
    slot = np.empty(E, np.int64)
    slot[order] = within + np.where(ks % 2 == 1, cnt_a[ks // 2], 0)

    # build-time template parameters (uniform across cores by construction)
    chunks_p = int(np.ceil(cnt_tot.max() / 128))
    bnd = cnt_a // 128
    bmin, bmax = int(bnd.min()), int(bnd.max())
    capp = chunks_p * 128

    idx_arr = np.full(tot_reg * capp, -1, np.int16)
    idx_arr[region * capp + slot] = sloc.astype(np.int16)
    cnt_arr = np.ascontiguousarray(
        cnt_tot.reshape(N_CORES, 1, NGATH).astype(np.int32)
    )

    # dstv columns: per region, A-cols for chunks [0, bmax], B-cols for
    # chunks [bmin, chunks_p); -5 where the slot isn't the column's half
    ncols_a = bmax + 1
    nc_pb = ncols_a + (chunks_p - bmin)
    chunk = slot // 128
    pos = slot % 128
    colidx = np.where(half == 0, chunk, ncols_a + (chunk - bmin))
    dcol = region * nc_pb + colidx
    dst_arr = np.full((tot_reg * nc_pb, 128), -5.0, np.float32)
    dst_arr[dcol, pos] = dloc.astype(np.float32)

    # idx: logical slot i of a gather -> partition i%16, col i//16; tile 16->128
    idx_sb = (
        idx_arr.reshape(N_CORES, NGATH, capp // 16, 16)
        .transpose(0, 3, 1, 2)
        .reshape(N_CORES, 16, NGATH * (capp // 16))
    )
    idx_sb = np.ascontiguousarray(np.tile(idx_sb, (1, 8, 1)))
    # dstv: [core, 128 partitions, cols]
    dst_sb = np.ascontiguousarray(
        dst_arr.reshape(N_CORES, NGATH * nc_pb, 128).transpose(0, 2, 1)
    )
    return idx_sb, dst_sb, cnt_arr, chunks_p, bmin, bmax


def _run(x, src_idx, dst_idx, trace=False, trace_kwargs=None):
    import ml_dtypes
    from concourse import bass_utils

    bf16 = ml_dtypes.bfloat16
    x = np.ascontiguousarray(np.asarray(x, dtype=np.float32))
    idx_sb, dst_sb, cnt_arr, chunks_p, bmin, bmax = _preprocess(
        np.asarray(src_idx), np.asarray(dst_idx)
    )

    tkey = (chunks_p, bmin, bmax)
    if _cached.get("key") != tkey:
        _cached["nc"] = _build_program(*tkey)
        _cached["key"] = tkey
    nc = _cached["nc"]

    ncols_a = bmax + 1
    nc_pair = NBKT * (ncols_a + (chunks_p - bmin))
    x_bf = x.astype(bf16)
    iota = np.tile(
        np.arange(BLOCK, dtype=np.float32), (128, nc_pair, 1)
    ).astype(bf16)
    ident = np.zeros((128, BLOCK), dtype=np.float32)
    ident[np.arange(BLOCK), np.arange(BLOCK)] = 1.0
    ident = ident.astype(bf16)
    in_maps = []
    for c in range(N_CORES):
        in_maps.append(
            {
                "x": x_bf,
                "xresb": x_bf[c * NODES_PER_CORE : c * NODES_PER_CORE + NBLOCKS * BLOCK],
                "idx": idx_sb[c],
                "cnt": cnt_arr[c],
                "dstv": dst_sb[c],
                "iota": iota,
                "ident": ident,
            }
        )
    kw = dict(trace_kwargs or {})
    res = bass_utils.run_bass_kernel_spmd(
        nc, in_maps, core_ids=list(range(N_CORES)), trace=trace, **kw
    )
    out = np.concatenate([r["out"] for r in res.results], axis=0)
    return out, res


def kernel(x, src_idx, dst_idx):
    out, _ = _run(x, src_idx, dst_idx)
    return out


# revision 16
# speedup vs baseline: 1.1079x; 1.1079x over previous
"""HGNN message passing (gather + segment_sum + residual) on 8 trn2 cores.

out = x + segment_sum(x[src_idx], dst_idx, num_segments=N)

Strategy (node-sharded accumulation, no collectives):
  - dst nodes sharded across 8 cores (12500 nodes each); each core owns the
    edges targeting its node range and produces its [12500, 128] output slice.
  - Nodes are processed in GROUPS of 4 blocks of 125. Edges of a group are
    bucketed by src//25000 (4 buckets, int16 gather-offset reach) and packed
    tightly, block-major, with a -1 tail the Q7 gather kernel never touches
    (the count register carries the exact edge count, so descriptor
    generation is O(edges) with no padding); big multi-block gathers
    amortize the per-instruction Q7 overhead that every GpSimd core pays.
  - bf16 x rows (256B) are fetched with gpsimd dma_gather across the 4 SWDGE
    queues (4 Q7 core-pairs generating descriptors in parallel);
    single_packet=False keeps each SDMA packet within hardware limits.
  - Per (group, bucket), ONE fused DVE is_equal builds the one-hot matrices
    (dstv column broadcast against a static iota image); the segment-sum is
    a sum of bf16 one-hot matmuls accumulated in PSUM, one PSUM tile per
    block. A chunk whose slot range can straddle a block boundary gets one
    matmul per candidate block (the host writes -5 into the other blocks'
    dstv so the one-hot is zero there). The residual enters the same PSUM
    accumulation as an identity-matrix matmul against the bf16 x row block,
    and the Scalar (ACT) engine copies PSUM to SBUF for the output DMA.

All cores run one SPMD program; per-core data differences live entirely in
the input tensors. The matmul template (chunk count, per-block chunk spans)
is computed from the actual edge data at build time, uniform across cores.
"""
import os

import numpy as np

N_NODES = 100000
D = 128
N_CORES = 8
NODES_PER_CORE = N_NODES // N_CORES  # 12500
BLOCK = 125
NBLOCKS = NODES_PER_CORE // BLOCK  # 100
if os.environ.get("KERNEL_NBLOCKS"):  # debug-only scale-down (multiple of 4)
    NBLOCKS = int(os.environ["KERNEL_NBLOCKS"])
GROUP = 4
NGROUPS = NBLOCKS // GROUP
NBKT = 4
SRC_CHUNK = N_NODES // NBKT  # 25000
NGATH = NGROUPS * NBKT  # gathers per core
STAGE_BUFS = 8

_cached = {}


def _build_program(chunks_p, los, his):
    """chunks_p: slots per (group,bucket) gather / 128; block h of a group
    only ever has edges in chunks [los[h], his[h]] (host-verified)."""
    from concourse import bacc, mybir, library_config
    import concourse.tile as tile

    capp = chunks_p * 128
    spans = [his[h] - los[h] + 1 for h in range(GROUP)]
    col_base = np.concatenate([[0], np.cumsum(spans)]).astype(int)
    nc_pb = int(col_base[-1])  # dstv/onehot columns per (group, bucket)
    idx_cols = NGATH * (capp // 16)

    nc = bacc.Bacc("TRN2", debug=False, num_swdge_queues=4)
    f32 = mybir.dt.float32
    bf16 = mybir.dt.bfloat16
    x_t = nc.dram_tensor("x", [N_NODES, D], bf16, kind="ExternalInput")
    xresb_t = nc.dram_tensor("xresb", [NBLOCKS * BLOCK, D], bf16, kind="ExternalInput")
    idx_t = nc.dram_tensor("idx", [128, idx_cols], mybir.dt.int16, kind="ExternalInput")
    cnt_t = nc.dram_tensor("cnt", [1, NGATH], mybir.dt.int32, kind="ExternalInput")
    dstv_t = nc.dram_tensor(
        "dstv", [128, NGATH * nc_pb], f32, kind="ExternalInput"
    )
    iota_t = nc.dram_tensor("iota", [128, nc_pb, BLOCK], bf16, kind="ExternalInput")
    ident_t = nc.dram_tensor("ident", [128, BLOCK], bf16, kind="ExternalInput")
    out_t = nc.dram_tensor("out", [NBLOCKS * BLOCK, D], f32, kind="ExternalOutput")

    with tile.TileContext(nc) as tc:
        with (
            tc.tile_pool(name="consts", bufs=1) as constp,
            tc.tile_pool(name="stage", bufs=STAGE_BUFS) as stagep,
            tc.tile_pool(name="oh", bufs=6) as ohp,
            tc.tile_pool(name="psum", bufs=4, space="PSUM") as psump,
            tc.tile_pool(name="resid", bufs=4) as residp,
            tc.tile_pool(name="osb", bufs=4) as osbp,
        ):
            nc.gpsimd.load_library(library_config.mlp)
            idx_sb = constp.tile([128, idx_cols], mybir.dt.int16)
            nc.sync.dma_start(idx_sb[:], idx_t[:])
            cnt_sb = constp.tile([1, NGATH], mybir.dt.int32)
            nc.sync.dma_start(cnt_sb[:], cnt_t[:])
            cnt_regs = [nc.gpsimd.alloc_register(f"cnt{k}") for k in range(NBKT)]
            dstv_sb = constp.tile([128, NGATH * nc_pb], f32)
            nc.sync.dma_start(dstv_sb[:], dstv_t[:])
            iota_sb = constp.tile([128, nc_pb, BLOCK], bf16)
            nc.sync.dma_start(iota_sb[:], iota_t[:])
            ident_sb = constp.tile([128, BLOCK], bf16)
            nc.sync.dma_start(ident_sb[:], ident_t[:])

            # zero staging/residual once: stale SBUF may hold NaN bit
            # patterns, and NaN * 0 would poison the PSUM accumulation
            for _ in range(STAGE_BUFS):
                stage = stagep.tile([128, chunks_p, D], bf16)
                nc.vector.memset(stage[:], 0.0)
            resid_bufs = []
            for _ in range(4):
                resid = residp.tile([128, D], bf16)
                nc.vector.memset(resid[:], 0.0)
                resid_bufs.append(resid)

            for grp in range(NGROUPS):
                stages_g = []
                ohbs_g = []
                for k in range(NBKT):
                    g = grp * NBKT + k
                    stage = stagep.tile([128, chunks_p, D], bf16)
                    nc.gpsimd.reg_load(cnt_regs[k], cnt_sb[:1, g : g + 1])
                    nc.gpsimd.dma_gather(
                        stage[:],
                        x_t[k * SRC_CHUNK : (k + 1) * SRC_CHUNK, :],
                        idx_sb[:, g * (capp // 16) : (g + 1) * (capp // 16)],
                        capp,
                        cnt_regs[k],
                        D,
                        single_packet=False,
                        queue_num=k,
                    )
                    stages_g.append(stage)
                    ohb = ohp.tile([128, nc_pb, BLOCK], bf16)
                    dstv_b = (
                        dstv_sb[:, g * nc_pb : (g + 1) * nc_pb]
                        .unsqueeze(2)
                        .broadcast_to([128, nc_pb, BLOCK])
                    )
                    nc.vector.tensor_tensor(
                        ohb[:], dstv_b, iota_sb[:], mybir.AluOpType.is_equal
                    )
                    ohbs_g.append(ohb)

                for h in range(GROUP):
                    b = grp * GROUP + h
                    resid = resid_bufs[b % 4]
                    nc.sync.dma_start(
                        resid[:BLOCK], xresb_t[b * BLOCK : (b + 1) * BLOCK, :]
                    )
                    psum = psump.tile([BLOCK, D], f32, space="PSUM")
                    nc.tensor.matmul(
                        out=psum[:],
                        lhsT=ident_sb[:],
                        rhs=resid[:],
                        start=True,
                        stop=False,
                    )
                    ncols_h = his[h] - los[h] + 1
                    for k in range(NBKT):
                        for i in range(ncols_h):
                            nc.tensor.matmul(
                                out=psum[:],
                                lhsT=ohbs_g[k][:, int(col_base[h]) + i, :],
                                rhs=stages_g[k][:, los[h] + i, :],
                                start=False,
                                stop=(k == NBKT - 1 and i == ncols_h - 1),
                            )
                    osb = osbp.tile([BLOCK, D], f32)
                    nc.scalar.copy(osb[:], psum[:])
                    nc.sync.dma_start(out_t[b * BLOCK : (b + 1) * BLOCK, :], osb[:])

    nc.compile()
    return nc


def _preprocess(src, dst):
    """Pack edges into tight per-(group,bucket) gather regions; build the idx
    image, exact counts, and the dstv one-hot source columns."""
    src = src.astype(np.int64)
    dst = dst.astype(np.int64)
    if NBLOCKS < NODES_PER_CORE // BLOCK:  # debug: drop edges past the cut
        keep = (dst % NODES_PER_CORE) // BLOCK < NBLOCKS
        src, dst = src[keep], dst[keep]
    E = src.shape[0]
    core = dst // NODES_PER_CORE
    blk = (dst % NODES_PER_CORE) // BLOCK
    half = blk % GROUP
    grp = blk // GROUP
    dloc = (dst % NODES_PER_CORE) % BLOCK
    bkt = src // SRC_CHUNK
    sloc = src % SRC_CHUNK
    region = (core * NGROUPS + grp) * NBKT + bkt  # gather region id
    tot_reg = N_CORES * NGATH

    key = region * GROUP + half
    order = np.argsort(key, kind="stable")
    ks = key[order]
    counts_h = np.bincount(key, minlength=tot_reg * GROUP)
    starts_h = np.zeros(tot_reg * GROUP + 1, np.int64)
    np.cumsum(counts_h, out=starts_h[1:])
    within = np.arange(E, dtype=np.int64) - starts_h[ks]

    ch = counts_h.reshape(tot_reg, GROUP)
    pref = np.zeros((tot_reg, GROUP + 1), np.int64)
    np.cumsum(ch, axis=1, out=pref[:, 1:])
    cnt_tot = pref[:, GROUP]
    # slot within region: block-major packing
    slot = np.empty(E, np.int64)
    slot[order] = within + pref[ks // GROUP, ks % GROUP]

    # build-time template parameters (uniform across cores by construction)
    chunks_p = int(np.ceil(cnt_tot.max() / 128))
    los = tuple(int(v) for v in (pref[:, :GROUP] // 128).min(axis=0))
    his = tuple(
        int(v) for v in (np.maximum(pref[:, 1:] - 1, pref[:, :GROUP]) // 128).max(axis=0)
    )
    capp = chunks_p * 128

    idx_arr = np.full(tot_reg * capp, -1, np.int16)
    idx_arr[region * capp + slot] = sloc.astype(np.int16)
    cnt_arr = np.ascontiguousarray(
        cnt_tot.reshape(N_CORES, 1, NGATH).astype(np.int32)
    )

    # dstv columns: per region, block h covers chunks [los[h], his[h]];
    # -5 where the slot isn't the column's block
    spans = [his[h] - los[h] + 1 for h in range(GROUP)]
    col_base = np.concatenate([[0], np.cumsum(spans)]).astype(np.int64)
    nc_pb = int(col_base[-1])
    chunk = slot // 128
    pos = slot % 128
    colidx = col_base[half] + (chunk - np.asarray(los)[half])
    dcol = region * nc_pb + colidx
    dst_arr = np.full((tot_reg * nc_pb, 128), -5.0, np.float32)
    dst_arr[dcol, pos] = dloc.astype(np.float32)

    # idx: logical slot i of a gather -> partition i%16, col i//16; tile 16->128
    idx_sb = (
        idx_arr.reshape(N_CORES, NGATH, capp // 16, 16)
        .transpose(0, 3, 1, 2)
        .reshape(N_CORES, 16, NGATH * (capp // 16))
    )
    idx_sb = np.ascontiguousarray(np.tile(idx_sb, (1, 8, 1)))
    # dstv: [core, 128 partitions, cols]
    dst_sb = np.ascontiguousarray(
        dst_arr.reshape(N_CORES, NGATH * nc_pb, 128).transpose(0, 2, 1)
    )
    return idx_sb, dst_sb, cnt_arr, chunks_p, los, his


def _run(x, src_idx, dst_idx, trace=False, trace_kwargs=None):
    import ml_dtypes
    from concourse import bass_utils

    bf16 = ml_dtypes.bfloat16
    x = np.ascontiguousarray(np.asarray(x, dtype=np.float32))
    idx_sb, dst_sb, cnt_arr, chunks_p, los, his = _preprocess(
        np.asarray(src_idx), np.asarray(dst_idx)
    )

    tkey = (chunks_p, los, his)
    if _cached.get("key") != tkey:
        _cached["nc"] = _build_program(chunks_p, los, his)
        _cached["key"] = tkey
    nc = _cached["nc"]

    nc_pb = sum(his[h] - los[h] + 1 for h in range(GROUP))
    x_bf = x.astype(bf16)
    iota = np.tile(
        np.arange(BLOCK, dtype=np.float32), (128, nc_pb, 1)
    ).astype(bf16)
    ident = np.zeros((128, BLOCK), dtype=np.float32)
    ident[np.arange(BLOCK), np.arange(BLOCK)] = 1.0
    ident = ident.astype(bf16)
    in_maps = []
    for c in range(N_CORES):
        in_maps.append(
            {
                "x": x_bf,
                "xresb": x_bf[c * NODES_PER_CORE : c * NODES_PER_CORE + NBLOCKS * BLOCK],
                "idx": idx_sb[c],
                "cnt": cnt_arr[c],
                "dstv": dst_sb[c],
                "iota": iota,
                "ident": ident,
            }
        )
    kw = dict(trace_kwargs or {})
    res = bass_utils.run_bass_kernel_spmd(
        nc, in_maps, core_ids=list(range(N_CORES)), trace=trace, **kw
    )
    out = np.concatenate([r["out"] for r in res.results], axis=0)
    return out, res


def kernel(x, src_idx, dst_idx):
    out, _ = _run(x, src_idx, dst_idx)
    return out


# revision 17
# speedup vs baseline: 1.1370x; 1.0263x over previous
"""HGNN message passing (gather + segment_sum + residual) on 8 trn2 cores.

out = x + segment_sum(x[src_idx], dst_idx, num_segments=N)

Strategy (node-sharded accumulation, no collectives):
  - dst nodes sharded across 8 cores (12500 nodes each); each core owns the
    edges targeting its node range and produces its [12500, 128] output slice.
  - Nodes are processed in GROUPS of 4 blocks of 125. Edges of a group are
    bucketed by src//25000 (4 buckets, int16 gather-offset reach) and packed
    tightly, block-major, with a -1 tail the Q7 gather kernel never touches
    (the count register carries the exact edge count, so descriptor
    generation is O(edges) with no padding); big multi-block gathers
    amortize the per-instruction Q7 overhead that every GpSimd core pays.
  - bf16 x rows (256B) are fetched with gpsimd dma_gather across the 4 SWDGE
    queues (4 Q7 core-pairs generating descriptors in parallel);
    single_packet=False keeps each SDMA packet within hardware limits.
  - Per (group, bucket), ONE fused DVE is_equal builds the one-hot matrices
    (dstv column broadcast against a static iota image); the segment-sum is
    a sum of bf16 one-hot matmuls accumulated in PSUM, one PSUM tile per
    block. A chunk whose slot range can straddle a block boundary gets one
    matmul per candidate block (the host writes -5 into the other blocks'
    dstv so the one-hot is zero there). The residual enters the same PSUM
    accumulation as an identity-matrix matmul against the bf16 x row block,
    and the Scalar (ACT) engine copies PSUM to SBUF for the output DMA.

All cores run one SPMD program; per-core data differences live entirely in
the input tensors. The matmul template (chunk count, per-block chunk spans)
is computed from the actual edge data at build time, uniform across cores.
"""
import os

import numpy as np

N_NODES = 100000
D = 128
N_CORES = 8
NODES_PER_CORE = N_NODES // N_CORES  # 12500
BLOCK = 125
NBLOCKS = NODES_PER_CORE // BLOCK  # 100
if os.environ.get("KERNEL_NBLOCKS"):  # debug-only scale-down (multiple of 4)
    NBLOCKS = int(os.environ["KERNEL_NBLOCKS"])
GROUP = 4
NGROUPS = NBLOCKS // GROUP
NBKT = 4
SRC_CHUNK = N_NODES // NBKT  # 25000
NGATH = NGROUPS * NBKT  # gathers per core
STAGE_BUFS = 8

_cached = {}


def _build_program(chunks_p, los, his):
    """chunks_p: slots per (group,bucket) gather / 128; block h of a group
    only ever has edges in chunks [los[h], his[h]] (host-verified)."""
    from concourse import bacc, mybir, library_config
    import concourse.tile as tile

    capp = chunks_p * 128
    spans = [his[h] - los[h] + 1 for h in range(GROUP)]
    col_base = np.concatenate([[0], np.cumsum(spans)]).astype(int)
    nc_pb = int(col_base[-1])  # dstv/onehot columns per (group, bucket)
    idx_cols = NGATH * (capp // 16)

    nc = bacc.Bacc("TRN2", debug=False, num_swdge_queues=4)
    f32 = mybir.dt.float32
    bf16 = mybir.dt.bfloat16
    x_t = nc.dram_tensor("x", [N_NODES, D], bf16, kind="ExternalInput")
    xresb_t = nc.dram_tensor("xresb", [NBLOCKS * BLOCK, D], bf16, kind="ExternalInput")
    idx_t = nc.dram_tensor("idx", [128, idx_cols], mybir.dt.int16, kind="ExternalInput")
    cnt_t = nc.dram_tensor("cnt", [1, NGATH], mybir.dt.int32, kind="ExternalInput")
    dstv_t = nc.dram_tensor(
        "dstv", [128, NGATH * nc_pb], bf16, kind="ExternalInput"
    )
    iota_t = nc.dram_tensor("iota", [128, nc_pb, BLOCK], bf16, kind="ExternalInput")
    ident_t = nc.dram_tensor("ident", [128, BLOCK], bf16, kind="ExternalInput")
    out_t = nc.dram_tensor("out", [NBLOCKS * BLOCK, D], f32, kind="ExternalOutput")

    with tile.TileContext(nc) as tc:
        with (
            tc.tile_pool(name="consts", bufs=1) as constp,
            tc.tile_pool(name="stage", bufs=STAGE_BUFS) as stagep,
            tc.tile_pool(name="oh", bufs=8) as ohp,
            tc.tile_pool(name="psum", bufs=4, space="PSUM") as psump,
            tc.tile_pool(name="resid", bufs=4) as residp,
            tc.tile_pool(name="osb", bufs=4) as osbp,
        ):
            nc.gpsimd.load_library(library_config.mlp)
            idx_sb = constp.tile([128, idx_cols], mybir.dt.int16)
            nc.sync.dma_start(idx_sb[:], idx_t[:])
            cnt_sb = constp.tile([1, NGATH], mybir.dt.int32)
            nc.sync.dma_start(cnt_sb[:], cnt_t[:])
            cnt_regs = [nc.gpsimd.alloc_register(f"cnt{k}") for k in range(NBKT)]
            dstv_sb = constp.tile([128, NGATH * nc_pb], bf16)
            nc.sync.dma_start(dstv_sb[:], dstv_t[:])
            iota_sb = constp.tile([128, nc_pb, BLOCK], bf16)
            nc.sync.dma_start(iota_sb[:], iota_t[:])
            ident_sb = constp.tile([128, BLOCK], bf16)
            nc.sync.dma_start(ident_sb[:], ident_t[:])

            # zero staging/residual once: stale SBUF may hold NaN bit
            # patterns, and NaN * 0 would poison the PSUM accumulation
            for _ in range(STAGE_BUFS):
                stage = stagep.tile([128, chunks_p, D], bf16)
                nc.vector.memset(stage[:], 0.0)
            resid_bufs = []
            for _ in range(4):
                resid = residp.tile([128, D], bf16)
                nc.vector.memset(resid[:], 0.0)
                resid_bufs.append(resid)

            for grp in range(NGROUPS):
                stages_g = []
                ohbs_g = []
                for k in range(NBKT):
                    g = grp * NBKT + k
                    stage = stagep.tile([128, chunks_p, D], bf16)
                    nc.gpsimd.reg_load(cnt_regs[k], cnt_sb[:1, g : g + 1])
                    nc.gpsimd.dma_gather(
                        stage[:],
                        x_t[k * SRC_CHUNK : (k + 1) * SRC_CHUNK, :],
                        idx_sb[:, g * (capp // 16) : (g + 1) * (capp // 16)],
                        capp,
                        cnt_regs[k],
                        D,
                        single_packet=False,
                        queue_num=k,
                    )
                    stages_g.append(stage)
                    ohb = ohp.tile([128, nc_pb, BLOCK], bf16)
                    dstv_b = (
                        dstv_sb[:, g * nc_pb : (g + 1) * nc_pb]
                        .unsqueeze(2)
                        .broadcast_to([128, nc_pb, BLOCK])
                    )
                    nc.vector.tensor_tensor(
                        ohb[:], dstv_b, iota_sb[:], mybir.AluOpType.is_equal
                    )
                    ohbs_g.append(ohb)

                for h in range(GROUP):
                    b = grp * GROUP + h
                    resid = resid_bufs[b % 4]
                    nc.sync.dma_start(
                        resid[:BLOCK], xresb_t[b * BLOCK : (b + 1) * BLOCK, :]
                    )
                    psum = psump.tile([BLOCK, D], f32, space="PSUM")
                    nc.tensor.matmul(
                        out=psum[:],
                        lhsT=ident_sb[:],
                        rhs=resid[:],
                        start=True,
                        stop=False,
                    )
                    ncols_h = his[h] - los[h] + 1
                    for k in range(NBKT):
                        for i in range(ncols_h):
                            nc.tensor.matmul(
                                out=psum[:],
                                lhsT=ohbs_g[k][:, int(col_base[h]) + i, :],
                                rhs=stages_g[k][:, los[h] + i, :],
                                start=False,
                                stop=(k == NBKT - 1 and i == ncols_h - 1),
                            )
                    osb = osbp.tile([BLOCK, D], f32)
                    nc.scalar.copy(osb[:], psum[:])
                    nc.sync.dma_start(out_t[b * BLOCK : (b + 1) * BLOCK, :], osb[:])

    nc.compile()
    return nc


def _preprocess(src, dst):
    """Pack edges into tight per-(group,bucket) gather regions; build the idx
    image, exact counts, and the dstv one-hot source columns."""
    src = src.astype(np.int64)
    dst = dst.astype(np.int64)
    if NBLOCKS < NODES_PER_CORE // BLOCK:  # debug: drop edges past the cut
        keep = (dst % NODES_PER_CORE) // BLOCK < NBLOCKS
        src, dst = src[keep], dst[keep]
    E = src.shape[0]
    core = dst // NODES_PER_CORE
    blk = (dst % NODES_PER_CORE) // BLOCK
    half = blk % GROUP
    grp = blk // GROUP
    dloc = (dst % NODES_PER_CORE) % BLOCK
    bkt = src // SRC_CHUNK
    sloc = src % SRC_CHUNK
    region = (core * NGROUPS + grp) * NBKT + bkt  # gather region id
    tot_reg = N_CORES * NGATH

    key = region * GROUP + half
    order = np.argsort(key, kind="stable")
    ks = key[order]
    counts_h = np.bincount(key, minlength=tot_reg * GROUP)
    starts_h = np.zeros(tot_reg * GROUP + 1, np.int64)
    np.cumsum(counts_h, out=starts_h[1:])
    within = np.arange(E, dtype=np.int64) - starts_h[ks]

    ch = counts_h.reshape(tot_reg, GROUP)
    pref = np.zeros((tot_reg, GROUP + 1), np.int64)
    np.cumsum(ch, axis=1, out=pref[:, 1:])
    cnt_tot = pref[:, GROUP]
    # slot within region: block-major packing
    slot = np.empty(E, np.int64)
    slot[order] = within + pref[ks // GROUP, ks % GROUP]

    # build-time template parameters (uniform across cores by construction)
    chunks_p = int(np.ceil(cnt_tot.max() / 128))
    los = tuple(int(v) for v in (pref[:, :GROUP] // 128).min(axis=0))
    his = tuple(
        int(v) for v in (np.maximum(pref[:, 1:] - 1, pref[:, :GROUP]) // 128).max(axis=0)
    )
    capp = chunks_p * 128

    idx_arr = np.full(tot_reg * capp, -1, np.int16)
    idx_arr[region * capp + slot] = sloc.astype(np.int16)
    cnt_arr = np.ascontiguousarray(
        cnt_tot.reshape(N_CORES, 1, NGATH).astype(np.int32)
    )

    # dstv columns: per region, block h covers chunks [los[h], his[h]];
    # -5 where the slot isn't the column's block
    spans = [his[h] - los[h] + 1 for h in range(GROUP)]
    col_base = np.concatenate([[0], np.cumsum(spans)]).astype(np.int64)
    nc_pb = int(col_base[-1])
    chunk = slot // 128
    pos = slot % 128
    colidx = col_base[half] + (chunk - np.asarray(los)[half])
    dcol = region * nc_pb + colidx
    dst_arr = np.full((tot_reg * nc_pb, 128), -5.0, np.float32)
    dst_arr[dcol, pos] = dloc.astype(np.float32)

    # idx: logical slot i of a gather -> partition i%16, col i//16; tile 16->128
    idx_sb = (
        idx_arr.reshape(N_CORES, NGATH, capp // 16, 16)
        .transpose(0, 3, 1, 2)
        .reshape(N_CORES, 16, NGATH * (capp // 16))
    )
    idx_sb = np.ascontiguousarray(np.tile(idx_sb, (1, 8, 1)))
    # dstv: [core, 128 partitions, cols]
    dst_sb = np.ascontiguousarray(
        dst_arr.reshape(N_CORES, NGATH * nc_pb, 128).transpose(0, 2, 1)
    )
    return idx_sb, dst_sb, cnt_arr, chunks_p, los, his


def _run(x, src_idx, dst_idx, trace=False, trace_kwargs=None):
    import ml_dtypes
    from concourse import bass_utils

    bf16 = ml_dtypes.bfloat16
    x = np.ascontiguousarray(np.asarray(x, dtype=np.float32))
    idx_sb, dst_sb, cnt_arr, chunks_p, los, his = _preprocess(
        np.asarray(src_idx), np.asarray(dst_idx)
    )
    dst_sb = dst_sb.astype(bf16)

    tkey = (chunks_p, los, his)
    if _cached.get("key") != tkey:
        _cached["nc"] = _build_program(chunks_p, los, his)
        _cached["key"] = tkey
    nc = _cached["nc"]

    nc_pb = sum(his[h] - los[h] + 1 for h in range(GROUP))
    x_bf = x.astype(bf16)
    iota = np.tile(
        np.arange(BLOCK, dtype=np.float32), (128, nc_pb, 1)
    ).astype(bf16)
    ident = np.zeros((128, BLOCK), dtype=np.float32)
    ident[np.arange(BLOCK), np.arange(BLOCK)] = 1.0
    ident = ident.astype(bf16)
    in_maps = []
    for c in range(N_CORES):
        in_maps.append(
            {
                "x": x_bf,
                "xresb": x_bf[c * NODES_PER_CORE : c * NODES_PER_CORE + NBLOCKS * BLOCK],
                "idx": idx_sb[c],
                "cnt": cnt_arr[c],
                "dstv": dst_sb[c],
                "iota": iota,
                "ident": ident,
            }
        )
    kw = dict(trace_kwargs or {})
    res = bass_utils.run_bass_kernel_spmd(
        nc, in_maps, core_ids=list(range(N_CORES)), trace=trace, **kw
    )
    out = np.concatenate([r["out"] for r in res.results], axis=0)
    return out, res


def kernel(x, src_idx, dst_idx):
    out, _ = _run(x, src_idx, dst_idx)
    return out
